# revision 1
# baseline (speedup 1.0000x reference)
"""Transformer encoder layer (LN -> MHA -> residual -> LN -> MLP -> residual)
on 8 Trainium2 NeuronCores.

Sharding: token-parallel over the 4096 (batch*seq) tokens, 512 query-tokens
per core; the 4 cores sharing a batch each redundantly compute the full
2048-token K/V for that batch, so no collectives are needed.

On-chip layout: activations are kept feature-major ("transposed", [d, token])
so every matmul contracts along the partition dim with weights in natural
[d_in, d_out] layout.  Softmax is computed unnormalized (scores are bounded,
so plain exp is numerically safe and algebraically identical); the denominator
comes for free from a ones-column appended to V, and the division is applied
in place to the tiny per-head attention accumulator.

LayerNorm gains/biases are folded into the following projections on the host
(exact algebra: (g*xhat+b) @ W = xhat @ (diag(g) W) + b @ W).
"""

import numpy as np

import concourse.bass as bass
import concourse.mybir as mybir
from concourse import bacc
from concourse.tile import TileContext
from concourse.bass_utils import run_bass_kernel_spmd
from concourse.masks import make_identity

F32 = mybir.dt.float32
F32R = mybir.dt.float32r
MMDT = F32R  # dtype for matmul operands (float32r = full-rate PE)
AF = mybir.ActivationFunctionType
ALU = mybir.AluOpType

B, S, D = 2, 2048, 1024
H, HD = 16, 64
DFF = 4 * D
NCORES = 8
QT = 512           # query tokens per core
NCHUNK = S // 512  # kv chunks of 512 tokens
EPS = 1e-5


def _ln_bcast_transpose(nc, lnp, psT, psS, bcp, ident, eps, ones128, x_dram, xT_dram, col0, hT):
    """LayerNorm 512 tokens: stats from token-major x tiles; normalization is
    applied in transposed space to x^T (DMA'd from host-prepared layout) via
    PE rank-1 broadcast of the per-token (-mu*rstd, rstd) rows."""
    mr_row = lnp.tile([1, 512], F32, tag="mr_row")
    rs_row = lnp.tile([1, 512], F32, tag="rs_row")
    for st in range(4):
        xt = lnp.tile([128, D], F32, tag="ln_x")
        nc.sync.dma_start(out=xt, in_=x_dram[col0 + st * 128:col0 + (st + 1) * 128, :])
        stats = lnp.tile([128, 2, 6], F32, tag="ln_st")
        nc.vector.bn_stats(stats[:, 0, :], xt[:, 0:512])
        nc.vector.bn_stats(stats[:, 1, :], xt[:, 512:1024])
        mv = lnp.tile([128, 2], F32, tag="ln_mv")
        nc.vector.bn_aggr(mv, stats)
        sd = lnp.tile([128, 1], F32, tag="ln_sd")
        nc.scalar.activation(sd, mv[:, 1:2], AF.Sqrt, bias=eps[:, 0:1])
        mr = lnp.tile([128, 2], F32, tag="ln_mr")
        nc.vector.reciprocal(mr[:, 1:2], sd)
        # mr[:,0] = -mu*rstd
        nc.vector.tensor_scalar(mr[:, 0:1], mv[:, 0:1], mr[:, 1:2], -1.0, ALU.mult, ALU.mult)
        pst = psT.tile([128, 128], F32, tag="tp")
        nc.tensor.transpose(pst[0:1, :], mr[:, 0:1], ident)
        nc.vector.tensor_copy(mr_row[:, st * 128:(st + 1) * 128], pst[0:1, :])
        pst2 = psT.tile([128, 128], F32, tag="tp")
        nc.tensor.transpose(pst2[0:1, :], mr[:, 1:2], ident)
        nc.vector.tensor_copy(rs_row[:, st * 128:(st + 1) * 128], pst2[0:1, :])
    # broadcast rows across 128 partitions via rank-1 matmuls
    mr_ps = psS.tile([128, 512], F32, tag="psS")
    nc.tensor.matmul(mr_ps, ones128, mr_row, start=True, stop=True)
    mr_bc = bcp.tile([128, 512], F32, tag="mr")
    nc.vector.tensor_copy(mr_bc, mr_ps)
    rs_ps = psS.tile([128, 512], F32, tag="psS")
    nc.tensor.matmul(rs_ps, ones128, rs_row, start=True, stop=True)
    rs_bc = bcp.tile([128, 512], F32, tag="rs")
    nc.vector.tensor_copy(rs_bc, rs_ps)
    # hT[dt] = xT[dt]*rs + mr  (in place over the DMA'd x^T bits)
    for dt in range(8):
        nc.sync.dma_start(
            out=hT[:, dt, :],
            in_=xT_dram[dt * 128:(dt + 1) * 128, col0:col0 + 512],
        )
        nc.vector.tensor_mul(hT[:, dt, :], hT[:, dt, :], rs_bc)
        nc.vector.tensor_add(hT[:, dt, :], hT[:, dt, :], mr_bc)


def _ln_transpose(nc, lnp, psT, ident, eps, x_src, hT, from_sbuf=False):
    """LayerNorm 512 tokens and write the transposed [d, token] result into
    hT ([128, 8, 512]).  x_src: DRAM AP rows [512, D] or SBUF tile view
    [128, 4, D]."""
    for st in range(4):
        if from_sbuf:
            xt = x_src[:, st, :]
        else:
            xt = lnp.tile([128, D], F32, tag="ln_x")
            nc.sync.dma_start(out=xt, in_=x_src[st * 128:(st + 1) * 128, :])
        stats = lnp.tile([128, 2, 6], F32, tag="ln_st")
        nc.vector.bn_stats(stats[:, 0, :], xt[:, 0:512])
        nc.vector.bn_stats(stats[:, 1, :], xt[:, 512:1024])
        mv = lnp.tile([128, 2], F32, tag="ln_mv")
        nc.vector.bn_aggr(mv, stats)
        sd = lnp.tile([128, 1], F32, tag="ln_sd")
        nc.scalar.activation(sd, mv[:, 1:2], AF.Sqrt, bias=eps[:, 0:1])
        rstd = lnp.tile([128, 1], F32, tag="ln_rs")
        nc.vector.reciprocal(rstd, sd)
        h = lnp.tile([128, D], F32, tag="ln_h")
        nc.vector.tensor_scalar(h, xt, mv[:, 0:1], rstd[:, 0:1], ALU.subtract, ALU.mult)
        for dt in range(8):
            pst = psT.tile([128, 128], F32, tag="tp")
            nc.tensor.transpose(pst, h[:, dt * 128:(dt + 1) * 128], ident)
            nc.vector.tensor_copy(hT[:, dt, st * 128:(st + 1) * 128], pst)


def _build():
    nc = bacc.Bacc(None, target_bir_lowering=False)

    XB = nc.declare_dram_parameter("xb", [S, D], F32, isOutput=False)
    XQ = nc.declare_dram_parameter("xq", [QT, D], F32, isOutput=False)
    XBT = nc.declare_dram_parameter("xbt", [D, S], MMDT, isOutput=False)
    XQT = nc.declare_dram_parameter("xqt", [D, QT], MMDT, isOutput=False)
    WQ = nc.declare_dram_parameter("wq", [D, D], MMDT, isOutput=False)
    WK = nc.declare_dram_parameter("wk", [D, D], MMDT, isOutput=False)
    WV = nc.declare_dram_parameter("wv", [D, D], MMDT, isOutput=False)
    WO = nc.declare_dram_parameter("wo", [D, D], MMDT, isOutput=False)
    W1 = nc.declare_dram_parameter("w1", [D, DFF], MMDT, isOutput=False)
    W2 = nc.declare_dram_parameter("w2", [DFF, D], MMDT, isOutput=False)
    BQ = nc.declare_dram_parameter("bq", [D], F32, isOutput=False)
    BK = nc.declare_dram_parameter("bk", [D], F32, isOutput=False)
    BV = nc.declare_dram_parameter("bv", [D], F32, isOutput=False)
    BO = nc.declare_dram_parameter("bo", [D], F32, isOutput=False)
    B1 = nc.declare_dram_parameter("b1", [DFF], F32, isOutput=False)
    B2 = nc.declare_dram_parameter("b2", [D], F32, isOutput=False)
    Y = nc.declare_dram_parameter("y", [QT, D], F32, isOutput=True)

    with TileContext(nc) as tc:
        with (
            tc.tile_pool(name="const", bufs=1) as cpool,
            tc.tile_pool(name="accp", bufs=1) as accp,
        ):
            ident = cpool.tile([128, 128], F32)
            make_identity(nc, ident)
            eps = cpool.tile([128, 1], F32)
            nc.vector.memset(eps, EPS)
            ones64 = cpool.tile([1, 64], F32)
            nc.vector.memset(ones64, 1.0)
            ones128 = cpool.tile([1, 128], F32)
            nc.vector.memset(ones128, 1.0)
            bqT = cpool.tile([128, 8], F32)
            nc.sync.dma_start(out=bqT, in_=BQ[:].rearrange("(t p) -> p t", p=128))
            bkT = cpool.tile([128, 8], F32)
            nc.sync.dma_start(out=bkT, in_=BK[:].rearrange("(t p) -> p t", p=128))
            b1T = cpool.tile([128, 32], F32)
            nc.sync.dma_start(out=b1T, in_=B1[:].rearrange("(t p) -> p t", p=128))
            bv_bc = cpool.tile([128, D], F32)
            nc.sync.dma_start(out=bv_bc, in_=BV[:].partition_broadcast(128))
            bo_bc = cpool.tile([128, D], F32)
            nc.sync.dma_start(out=bo_bc, in_=BO[:].partition_broadcast(128))
            b2_bc = cpool.tile([128, D], F32)
            nc.sync.dma_start(out=b2_bc, in_=B2[:].partition_broadcast(128))

            acc = accp.tile([65, 16, 512], F32)  # unnormalized attn^T + denom row

            # ---- projections + attention, streamed over kv chunks ----
            with (
                tc.tile_pool(name="qp", bufs=1) as qp,
                tc.tile_pool(name="lnp", bufs=2) as lnp,
                tc.tile_pool(name="hTp", bufs=2) as hTp,
                tc.tile_pool(name="ktp", bufs=1) as ktp,
                tc.tile_pool(name="vp", bufs=2) as vp,
                tc.tile_pool(name="wsm", bufs=3) as wsm,
                tc.tile_pool(name="wvp", bufs=1) as wvp,
                tc.tile_pool(name="bcp", bufs=2) as bcp,
                tc.tile_pool(name="pp", bufs=6) as ppl,
                tc.tile_pool(name="psK", bufs=2, space="PSUM") as psK,
                tc.tile_pool(name="psV", bufs=1, space="PSUM") as psV,
                tc.tile_pool(name="psS", bufs=3, space="PSUM") as psS,
                tc.tile_pool(name="psA", bufs=1, space="PSUM") as psA,
                tc.tile_pool(name="psT", bufs=1, space="PSUM") as psT,
            ):
                Q_sb = qp.tile([128, 8, 512], MMDT)  # Q^T [hd, q]

                # Q projection from the core's own tokens
                hqT = hTp.tile([128, 8, 512], MMDT, tag="hT")
                _ln_bcast_transpose(nc, lnp, psT, psS, bcp, ident, eps, ones128, XQ, XQT, 0, hqT)
                for ht in range(8):
                    wcol = wsm.tile([128, 8, 128], MMDT, tag="w")
                    nc.sync.dma_start(
                        out=wcol,
                        in_=WQ[:, ht * 128:(ht + 1) * 128].rearrange(
                            "(t p) n -> p t n", p=128
                        ),
                    )
                    psq = psK.tile([128, 512], F32, tag="psK")
                    for dt in range(8):
                        nc.tensor.matmul(
                            psq, wcol[:, dt, :], hqT[:, dt, :],
                            start=(dt == 0), stop=(dt == 7),
                        )
                    nc.vector.tensor_scalar_add(Q_sb[:, ht, :], psq, bqT[:, ht:ht + 1])

                for kc in range(NCHUNK):
                    hT = hTp.tile([128, 8, 512], MMDT, tag="hT")
                    _ln_bcast_transpose(nc, lnp, psT, psS, bcp, ident, eps, ones128, XB, XBT, kc * 512, hT)

                    # K^T chunk [hd, 512]
                    KT = ktp.tile([128, 8, 512], MMDT, tag="KT")
                    for ht in range(8):
                        wcol = wsm.tile([128, 8, 128], MMDT, tag="w")
                        nc.sync.dma_start(
                            out=wcol,
                            in_=WK[:, ht * 128:(ht + 1) * 128].rearrange(
                                "(t p) n -> p t n", p=128
                            ),
                        )
                        psk = psK.tile([128, 512], F32, tag="psK")
                        for dt in range(8):
                            nc.tensor.matmul(
                                psk, wcol[:, dt, :], hT[:, dt, :],
                                start=(dt == 0), stop=(dt == 7),
                            )
                        nc.vector.tensor_scalar_add(KT[:, ht, :], psk, bkT[:, ht:ht + 1])

                    # V chunk, natural layout [token, head, hd] + ones column
                    V = vp.tile([128, 4, 16, 65], MMDT, tag="V")
                    nc.vector.memset(V[:, :, :, 64:65].bitcast(F32), 1.0)
                    for hc in range(2):
                        wv_sb = wvp.tile([128, 8, 512], MMDT, tag="wv")
                        nc.sync.dma_start(
                            out=wv_sb,
                            in_=WV[:, hc * 512:(hc + 1) * 512].rearrange(
                                "(t p) n -> p t n", p=128
                            ),
                        )
                        for st in range(4):
                            psv = psV.tile([128, 512], F32, tag="psV")
                            for dt in range(8):
                                nc.tensor.matmul(
                                    psv,
                                    hT[:, dt, st * 128:(st + 1) * 128],
                                    wv_sb[:, dt, :],
                                    start=(dt == 0),
                                    stop=(dt == 7),
                                )
                            nc.vector.tensor_add(
                                V[:, st, hc * 8:(hc + 1) * 8, 0:64],
                                psv.rearrange("p (h d) -> p h d", h=8),
                                bv_bc[:, hc * 512:(hc + 1) * 512].rearrange(
                                    "p (h d) -> p h d", h=8
                                ),
                            )

                    # attention for this kv chunk
                    for h in range(H):
                        ko = (h % 2) * 64
                        kj = h // 2
                        p_tiles = []
                        for kt in range(4):
                            pss = psS.tile([128, 512], F32, tag="psS")
                            nc.tensor.matmul(
                                pss,
                                KT[ko:ko + 64, kj, kt * 128:(kt + 1) * 128],
                                Q_sb[ko:ko + 64, kj, :],
                                start=True,
                                stop=True,
                            )
                            P = ppl.tile([128, 512], MMDT, tag="P")
                            nc.scalar.activation(P, pss, AF.Exp, scale=0.125)
                            p_tiles.append(P)
                        psa = psA.tile([65, 512], F32, tag="psA")
                        for kt in range(4):
                            nc.tensor.matmul(
                                psa, V[:, kt, h, :], p_tiles[kt],
                                start=(kt == 0), stop=(kt == 3),
                            )
                        if kc == 0:
                            nc.vector.tensor_copy(acc[:, h, :], psa)
                        else:
                            nc.vector.tensor_add(acc[:, h, :], acc[:, h, :], psa)

            # ---- softmax normalization + out-projection + residual ----
            with tc.tile_pool(name="x2p", bufs=1) as x2p:
              x2 = x2p.tile([128, 4, D], F32)  # post-attention residual stream
              with (
                tc.tile_pool(name="attnp", bufs=1) as attnp,
                tc.tile_pool(name="dsm", bufs=4) as dsm,
                tc.tile_pool(name="psRB", bufs=2, space="PSUM") as psRB,
                tc.tile_pool(name="xqp", bufs=1) as xqp,
                tc.tile_pool(name="dwo", bufs=6) as dwo,
                tc.tile_pool(name="dtmp", bufs=4) as dtmp,
                tc.tile_pool(name="psO", bufs=4, space="PSUM") as psO,
            ):
                attn128 = attnp.tile([128, 8, 512], MMDT)
                for h in range(H):
                    r = dsm.tile([1, 512], F32, tag="r")
                    nc.vector.reciprocal(r, acc[64:65, h, :])
                    rb_ps = psRB.tile([64, 512], F32, tag="rb")
                    nc.tensor.matmul(rb_ps, ones64, r, start=True, stop=True)
                    rb = dsm.tile([64, 512], F32, tag="rb_sb")
                    nc.scalar.copy(rb, rb_ps)
                    ko = (h % 2) * 64
                    nc.vector.tensor_mul(
                        attn128[ko:ko + 64, h // 2, :], acc[0:64, h, :], rb
                    )

                xq_sb = xqp.tile([128, 4, D], F32)
                nc.sync.dma_start(
                    out=xq_sb, in_=XQ[:].rearrange("(t p) n -> p t n", p=128)
                )
                for c in range(2):
                    po = [psO.tile([128, 512], F32, tag="psO", name=f"po{c}_{i}") for i in range(4)]
                    for j in range(8):
                        wot = dwo.tile([128, 512], MMDT, tag="wo")
                        nc.sync.dma_start(
                            out=wot,
                            in_=WO[j * 128:(j + 1) * 128, c * 512:(c + 1) * 512],
                        )
                        for qt in range(4):
                            nc.tensor.matmul(
                                po[qt], attn128[:, j, qt * 128:(qt + 1) * 128], wot,
                                start=(j == 0), stop=(j == 7),
                            )
                    for qt in range(4):
                        t1 = dtmp.tile([128, 512], F32, tag="t1")
                        nc.vector.tensor_add(
                            t1, po[qt], bo_bc[:, c * 512:(c + 1) * 512]
                        )
                        nc.vector.tensor_add(
                            x2[:, qt, c * 512:(c + 1) * 512],
                            t1,
                            xq_sb[:, qt, c * 512:(c + 1) * 512],
                        )

              # ---- LN2 + MLP + residual ----
              with (
                  tc.tile_pool(name="lnp2", bufs=2) as lnp2,
                  tc.tile_pool(name="h2p", bufs=1) as h2p,
                  tc.tile_pool(name="gp", bufs=1) as gp,
                  tc.tile_pool(name="wfp", bufs=6) as wfp,
                  tc.tile_pool(name="w2p", bufs=6) as w2p,
                  tc.tile_pool(name="yp", bufs=2) as yp,
              ):
                  h2T = h2p.tile([128, 8, 512], MMDT)
                  G = gp.tile([128, 32, 512], MMDT)
                  with (
                      tc.tile_pool(name="psT2", bufs=2, space="PSUM") as psT2,
                      tc.tile_pool(name="psF", bufs=4, space="PSUM") as psF,
                  ):
                      _ln_transpose(nc, lnp2, psT2, ident, eps, x2, h2T, from_sbuf=True)

                      # MLP1: gelu(h2 @ w1 + b1), transposed output [dff, q]
                      for ft in range(32):
                          w1c = wfp.tile([128, 8, 128], MMDT, tag="w1")
                          nc.sync.dma_start(
                              out=w1c,
                              in_=W1[:, ft * 128:(ft + 1) * 128].rearrange(
                                  "(t p) n -> p t n", p=128
                              ),
                          )
                          psf = psF.tile([128, 512], F32, tag="psF")
                          for dt in range(8):
                              nc.tensor.matmul(
                                  psf, w1c[:, dt, :], h2T[:, dt, :],
                                  start=(dt == 0), stop=(dt == 7),
                              )
                          nc.scalar.activation(
                              G[:, ft, :], psf, AF.Gelu, bias=b1T[:, ft:ft + 1]
                          )

                  # MLP2: y = G^T @ w2 + b2 + x2
                  with tc.tile_pool(name="psY", bufs=4, space="PSUM") as psY:
                    for c in range(2):
                      py = [psY.tile([128, 512], F32, tag="psY", name=f"py{c}_{i}") for i in range(4)]
                      for ft in range(32):
                          w2t = w2p.tile([128, 512], MMDT, tag="w2")
                          nc.sync.dma_start(
                              out=w2t,
                              in_=W2[ft * 128:(ft + 1) * 128, c * 512:(c + 1) * 512],
                          )
                          for qt in range(4):
                              nc.tensor.matmul(
                                  py[qt], G[:, ft, qt * 128:(qt + 1) * 128], w2t,
                                  start=(ft == 0), stop=(ft == 31),
                              )
                      for qt in range(4):
                          t1 = yp.tile([128, 512], F32, tag="yt1")
                          nc.vector.tensor_add(
                              t1, py[qt], b2_bc[:, c * 512:(c + 1) * 512]
                          )
                          yt = yp.tile([128, 512], F32, tag="yt2")
                          nc.vector.tensor_add(
                              yt, t1, x2[:, qt, c * 512:(c + 1) * 512]
                          )
                          nc.sync.dma_start(
                              out=Y[qt * 128:(qt + 1) * 128, c * 512:(c + 1) * 512],
                              in_=yt,
                          )

    nc.compile()
    return nc


_NC = None


def _get_nc():
    global _NC
    if _NC is None:
        _NC = _build()
    return _NC


def kernel(x, ln1_g, ln1_b, wq, bq, wk, bk, wv, bv, wo, bo, w1, b1, w2, b2, ln2_g, ln2_b):
    f32 = lambda a: np.ascontiguousarray(np.asarray(a, dtype=np.float32))
    x = f32(x)
    ln1_g, ln1_b = f32(ln1_g), f32(ln1_b)
    ln2_g, ln2_b = f32(ln2_g), f32(ln2_b)
    wq, wk, wv, wo = f32(wq), f32(wk), f32(wv), f32(wo)
    w1, w2 = f32(w1), f32(w2)
    bq, bk, bv, bo, b1, b2 = f32(bq), f32(bk), f32(bv), f32(bo), f32(b1), f32(b2)

    # Fold LayerNorm affine params into the following projections (exact).
    wq_e = f32(ln1_g[:, None] * wq)
    wk_e = f32(ln1_g[:, None] * wk)
    wv_e = f32(ln1_g[:, None] * wv)
    bq_e = f32(bq + ln1_b @ wq)
    bk_e = f32(bk + ln1_b @ wk)
    bv_e = f32(bv + ln1_b @ wv)
    w1_e = f32(ln2_g[:, None] * w1)
    b1_e = f32(b1 + ln2_b @ w1)

    common = {
        "wq": wq_e, "wk": wk_e, "wv": wv_e, "wo": wo,
        "w1": w1_e, "w2": w2,
        "bq": bq_e, "bk": bk_e, "bv": bv_e, "bo": bo,
        "b1": b1_e, "b2": b2,
    }
    in_maps = []
    for c in range(NCORES):
        b = c // 4
        qoff = (c % 4) * QT
        m = dict(common)
        m["xb"] = np.ascontiguousarray(x[b])
        m["xq"] = np.ascontiguousarray(x[b, qoff:qoff + QT])
        m["xbt"] = np.ascontiguousarray(x[b].T)
        m["xqt"] = np.ascontiguousarray(x[b, qoff:qoff + QT].T)
        in_maps.append(m)

    nc = _get_nc()
    res = run_bass_kernel_spmd(nc, in_maps, core_ids=list(range(NCORES)))

    y = np.empty((B, S, D), dtype=np.float32)
    for c in range(NCORES):
        b = c // 4
        qoff = (c % 4) * QT
        y[b, qoff:qoff + QT] = res.results[c]["y"]
    return y



# revision 13
# speedup vs baseline: 1.1416x; 1.1416x over previous
"""Transformer encoder layer (LN -> MHA -> residual -> LN -> MLP -> residual)
on 8 Trainium2 NeuronCores.

Sharding: token-parallel over the 4096 (batch*seq) tokens, 512 query-tokens
per core.  Each core computes K/V projections only for its OWN 512 tokens;
the full 2048-token K/V per batch is assembled with two AllGather
collectives (bf16, ~1 MB each) across the 4-core group sharing a batch.
Collectives run on TOPSP/SDMA and overlap with the Q projection.

On-chip layout: activations are kept feature-major ("transposed", [d, token])
so every matmul contracts along the partition dim.  Weights are pre-arranged
on the host so every weight DMA is one contiguous run per partition (DMA
descriptor count is the latency driver, not bytes).  Matmul operands are
bf16; accumulation stays fp32 in PSUM.  Softmax is computed unnormalized
(scores are bounded so plain exp is safe and algebraically identical); the
denominator comes from a ones-column interleaved into V before the gather,
and each head's attention accumulator stays resident in one PSUM bank
across all 16 k-tiles.  The score matmuls run one (wave, chunk) step ahead
of the attn@V matmuls so the exp's on the scalar engine pipeline behind
full-speed PE bursts.

LayerNorm gains/biases are folded into the following projections on the host
(exact algebra: (g*xhat+b) @ W = xhat @ (diag(g) W) + b @ W).
"""

import numpy as np
import ml_dtypes

import concourse.bass as bass
import concourse.mybir as mybir
from concourse import bacc
from concourse.tile import TileContext
from concourse.bass_utils import run_bass_kernel_spmd
from concourse.masks import make_identity

F32 = mybir.dt.float32
F32R = mybir.dt.float32r
BF16 = mybir.dt.bfloat16
AF = mybir.ActivationFunctionType
ALU = mybir.AluOpType

B, S, D = 2, 2048, 1024
H, HD = 16, 64
DFF = 4 * D
NCORES = 8
QT = 512            # query tokens per core
EPS = 1e-5
RG = [[0, 1, 2, 3], [4, 5, 6, 7]]  # replica groups (one per batch)


def _ln_stats(nc, lnp, eps, xt_a, xt_b):
    """bn stats over two [128, 512] token half-tiles -> (-mu*rstd, rstd)."""
    stats = lnp.tile([128, 2, 6], F32, tag="ln_st")
    nc.vector.bn_stats(stats[:, 0, :], xt_a)
    nc.vector.bn_stats(stats[:, 1, :], xt_b)
    mv = lnp.tile([128, 2], F32, tag="ln_mv")
    nc.vector.bn_aggr(mv, stats)
    sd = lnp.tile([128, 1], F32, tag="ln_sd")
    nc.scalar.activation(sd, mv[:, 1:2], AF.Sqrt, bias=eps[:, 0:1])
    rstd = lnp.tile([128, 1], F32, tag="ln_rs")
    nc.vector.reciprocal(rstd, sd)
    mr = lnp.tile([128, 2], F32R, tag="ln_mr")
    nc.vector.tensor_scalar(
        mr[:, 0:1], mv[:, 0:1], rstd, -1.0, ALU.mult, ALU.mult
    )
    nc.vector.tensor_copy(mr[:, 1:2], rstd)
    return mr


def _build():
    nc = bacc.Bacc(None, target_bir_lowering=False, num_devices=NCORES)

    XQ = nc.declare_dram_parameter("xq", [QT, D], F32, isOutput=False)
    XQT = nc.declare_dram_parameter("xqt", [D, QT], F32, isOutput=False)
    # host-prearranged weights: one contiguous run per partition per load
    WQR = nc.declare_dram_parameter("wqr", [8, 128, 8, 128], BF16, isOutput=False)
    WKR = nc.declare_dram_parameter("wkr", [8, 128, 8, 128], BF16, isOutput=False)
    WVR = nc.declare_dram_parameter("wvr", [2, 128, 8, 512], BF16, isOutput=False)
    WO = nc.declare_dram_parameter("wo", [D, D], BF16, isOutput=False)
    W1R = nc.declare_dram_parameter("w1r", [32, 128, 8, 128], BF16, isOutput=False)
    W2 = nc.declare_dram_parameter("w2", [DFF, D], BF16, isOutput=False)
    BQ = nc.declare_dram_parameter("bq", [D], F32, isOutput=False)
    BK = nc.declare_dram_parameter("bk", [D], F32, isOutput=False)
    BV = nc.declare_dram_parameter("bv", [D], F32, isOutput=False)
    BO = nc.declare_dram_parameter("bo", [D], F32, isOutput=False)
    B1 = nc.declare_dram_parameter("b1", [DFF], F32, isOutput=False)
    B2 = nc.declare_dram_parameter("b2", [D], F32, isOutput=False)
    Y = nc.declare_dram_parameter("y", [QT, D], F32, isOutput=True)

    with TileContext(nc) as tc:
        with (
            tc.tile_pool(name="const", bufs=1) as cpool,
            tc.tile_pool(name="dram", bufs=1, space="DRAM") as dpool,
            tc.tile_pool(name="accp", bufs=1) as accp,
            tc.tile_pool(name="x2p", bufs=1) as x2p,
        ):
            ident32 = cpool.tile([128, 128], F32)
            make_identity(nc, ident32)
            ident16 = cpool.tile([128, 128], BF16)
            nc.vector.tensor_copy(ident16, ident32)
            eps = cpool.tile([128, 1], F32)
            nc.vector.memset(eps, EPS)
            ones64 = cpool.tile([1, 64], BF16)
            nc.vector.memset(ones64, 1.0)
            ones128f = cpool.tile([1, 128], F32)
            nc.vector.memset(ones128f, 1.0)
            ones128 = cpool.tile([1, 128], F32R)
            nc.vector.tensor_copy(ones128, ones128f)
            bqT = cpool.tile([128, 8], F32)
            nc.sync.dma_start(out=bqT, in_=BQ[:].rearrange("(t p) -> p t", p=128))
            bkT = cpool.tile([128, 8], F32)
            nc.sync.dma_start(out=bkT, in_=BK[:].rearrange("(t p) -> p t", p=128))
            b1T = cpool.tile([128, 32], F32)
            nc.sync.dma_start(out=b1T, in_=B1[:].rearrange("(t p) -> p t", p=128))
            bv_bc = cpool.tile([128, D], F32)
            nc.sync.dma_start(out=bv_bc, in_=BV[:].partition_broadcast(128))
            bo_bc = cpool.tile([128, D], F32)
            nc.sync.dma_start(out=bo_bc, in_=BO[:].partition_broadcast(128))
            b2_bc = cpool.tile([128, D], F32)
            nc.sync.dma_start(out=b2_bc, in_=B2[:].partition_broadcast(128))

            # DRAM bounce buffers for the K/V AllGathers (flat, partition-major)
            kT_loc = dpool.tile([128, 8 * QT], BF16)
            v_loc = dpool.tile([128, 4 * H * 65], BF16)
            kT_full = dpool.tile([4, 128, 8 * QT], BF16)
            v_full = dpool.tile([4, 128, 4 * H * 65], BF16)

            acc = accp.tile([65, H, QT], BF16)  # unnormalized attn^T + denom
            x2 = x2p.tile([128, 4, D], F32)     # post-attention residual

            # ---- Phase 1: LN1 + K/V proj (own tokens) + gathers + Q proj ----
            with tc.tile_pool(name="qp", bufs=1) as qp:
              with (
                tc.tile_pool(name="kvloc", bufs=1) as kvp,
                tc.tile_pool(name="lnp", bufs=3) as lnp,
                tc.tile_pool(name="hTp", bufs=1) as hTp,
                tc.tile_pool(name="wsm", bufs=6) as wsm,
                tc.tile_pool(name="bcp", bufs=1) as bcp,
                tc.tile_pool(name="psT", bufs=2, space="PSUM") as psT,
                tc.tile_pool(name="psS1", bufs=1, space="PSUM") as psS1,
                tc.tile_pool(name="psK", bufs=3, space="PSUM") as psK,
              ):
                # -- LN1: stats in token space, apply in transposed space --
                hqT = hTp.tile([128, 8, QT], BF16)
                mr_row = lnp.tile([1, 512], F32R, tag="mr_row")
                rs_row = lnp.tile([1, 512], F32R, tag="rs_row")
                for st in range(4):
                    xt_a = lnp.tile([128, 512], F32, tag="ln_xa")
                    nc.sync.dma_start(
                        out=xt_a, in_=XQ[st * 128:(st + 1) * 128, 0:512])
                    xt_b = lnp.tile([128, 512], F32, tag="ln_xb")
                    nc.sync.dma_start(
                        out=xt_b, in_=XQ[st * 128:(st + 1) * 128, 512:1024])
                    mr = _ln_stats(nc, lnp, eps, xt_a, xt_b)
                    pst = psT.tile([128, 128], F32, tag="tp")
                    nc.tensor.transpose(pst[0:1, :], mr[:, 0:1].bitcast(F32),
                                        ident32)
                    nc.vector.tensor_copy(
                        mr_row[:, st * 128:(st + 1) * 128], pst[0:1, :])
                    pst2 = psT.tile([128, 128], F32, tag="tp")
                    nc.tensor.transpose(pst2[0:1, :], mr[:, 1:2].bitcast(F32),
                                        ident32)
                    nc.vector.tensor_copy(
                        rs_row[:, st * 128:(st + 1) * 128], pst2[0:1, :])
                # broadcast across partitions via rank-1 matmuls (f32r)
                mr_ps = psS1.tile([128, 512], F32, tag="psS")
                nc.tensor.matmul(mr_ps, ones128, mr_row, start=True, stop=True)
                mr_bc = bcp.tile([128, 512], F32, tag="mr")
                nc.vector.tensor_copy(mr_bc, mr_ps)
                rs_ps = psS1.tile([128, 512], F32, tag="psS")
                nc.tensor.matmul(rs_ps, ones128, rs_row, start=True, stop=True)
                rs_bc = bcp.tile([128, 512], F32, tag="rs")
                nc.vector.tensor_copy(rs_bc, rs_ps)
                for dt in range(8):
                    xtt = lnp.tile([128, 512], F32, tag="ln_xt")
                    nc.sync.dma_start(
                        out=xtt, in_=XQT[dt * 128:(dt + 1) * 128, :])
                    nc.vector.tensor_mul(xtt, xtt, rs_bc)
                    nc.vector.tensor_add(hqT[:, dt, :], xtt, mr_bc)

                # -- K proj -> feature-major [d, t] + bias, then gather --
                kloc_sb = kvp.tile([128, 8, QT], BF16)
                for ht in range(8):
                    wcol = wsm.tile([128, 8, 128], BF16, tag="w")
                    nc.sync.dma_start(out=wcol, in_=WKR[ht])
                    psk = psK.tile([128, 512], F32, tag="psK")
                    for dt in range(8):
                        nc.tensor.matmul(
                            psk, wcol[:, dt, :], hqT[:, dt, :],
                            start=(dt == 0), stop=(dt == 7),
                        )
                    nc.vector.tensor_scalar_add(
                        kloc_sb[:, ht, :], psk, bkT[:, ht:ht + 1]
                    )
                nc.gpsimd.dma_start(
                    out=kT_loc[:, :],
                    in_=kloc_sb[:].rearrange("p t n -> p (t n)"),
                )
                nc.gpsimd.collective_compute(
                    "AllGather", ALU.bypass, replica_groups=RG,
                    ins=[kT_loc.opt()], outs=[kT_full.opt()],
                )

                # -- V proj -> token-major [t, (h 65)] + bias + ones col --
                vloc_sb = kvp.tile([128, 4, H, 65], BF16)
                nc.vector.memset(vloc_sb[:, :, :, 64:65], 1.0)
                for hc in range(2):
                    wv_sb = wsm.tile([128, 8, 512], BF16, tag="wv",
                                     name=f"wv{hc}")
                    for dq in range(4):
                        nc.sync.dma_start(
                            out=wv_sb[:, dq * 2:(dq + 1) * 2, :],
                            in_=WVR[hc, :, dq * 2:(dq + 1) * 2, :],
                        )
                    for st in range(4):
                        psv = psK.tile([128, 512], F32, tag="psK")
                        for dt in range(8):
                            nc.tensor.matmul(
                                psv,
                                hqT[:, dt, st * 128:(st + 1) * 128],
                                wv_sb[:, dt, :],
                                start=(dt == 0), stop=(dt == 7),
                            )
                        nc.vector.tensor_add(
                            vloc_sb[:, st, hc * 8:(hc + 1) * 8, 0:64],
                            psv.rearrange("p (h d) -> p h d", h=8),
                            bv_bc[:, hc * 512:(hc + 1) * 512].rearrange(
                                "p (h d) -> p h d", h=8),
                        )
                nc.gpsimd.dma_start(
                    out=v_loc[:, :],
                    in_=vloc_sb[:].rearrange("p s h d -> p (s h d)"),
                )
                nc.gpsimd.collective_compute(
                    "AllGather", ALU.bypass, replica_groups=RG,
                    ins=[v_loc.opt()], outs=[v_full.opt()],
                )

                # -- Q proj (overlaps with the gathers) --
                Q_sb = qp.tile([128, 8, QT], BF16)
                for ht in range(8):
                    wcol = wsm.tile([128, 8, 128], BF16, tag="w")
                    nc.sync.dma_start(out=wcol, in_=WQR[ht])
                    psq = psK.tile([128, 512], F32, tag="psK")
                    for dt in range(8):
                        nc.tensor.matmul(
                            psq, wcol[:, dt, :], hqT[:, dt, :],
                            start=(dt == 0), stop=(dt == 7),
                        )
                    nc.vector.tensor_scalar_add(
                        Q_sb[:, ht, :], psq, bqT[:, ht:ht + 1]
                    )

              # ---- Phase 2+3: load gathered K/V, pipelined attention ----
              with (
                  tc.tile_pool(name="kvall", bufs=1) as kva,
                  tc.tile_pool(name="pp", bufs=36) as ppl,
                  tc.tile_pool(name="psS", bufs=4, space="PSUM") as psS,
                  tc.tile_pool(name="psA", bufs=4, space="PSUM") as psA,
              ):
                  KT_all = kva.tile([128, 4, 8, QT], BF16)
                  V_all = kva.tile([128, 4, 4, H, 65], BF16)
                  for g in range(4):
                      for half in range(2):
                          nc.sync.dma_start(
                              out=KT_all[:, g, half * 4:(half + 1) * 4, :],
                              in_=kT_full[g][:, half * 2048:(half + 1) * 2048]
                              .rearrange("p (t n) -> p t n", t=4),
                          )
                          nc.sync.dma_start(
                              out=V_all[:, g, half * 2:(half + 1) * 2, :, :],
                              in_=v_full[g][:, half * 2080:(half + 1) * 2080]
                              .rearrange("p (s h d) -> p s h d", s=2, h=H),
                          )

                  # per (wave, chunk) step: 16 score matmuls + 16 exps +
                  # 16 attn@V matmuls; scores run one step ahead
                  steps = [(w, g) for w in range(4) for g in range(4)]
                  psa = {}
                  p_tiles = {}

                  def emit_scores(w, g):
                      for h in range(w * 4, w * 4 + 4):
                          kj, ko = h // 2, (h % 2) * 64
                          for ktl in range(4):
                              pss = psS.tile([128, 512], F32, tag="psS",
                                             name=f"pss{h}_{g}_{ktl}")
                              nc.tensor.matmul(
                                  pss,
                                  KT_all[ko:ko + 64, g, kj,
                                         ktl * 128:(ktl + 1) * 128],
                                  Q_sb[ko:ko + 64, kj, :],
                                  start=True, stop=True,
                              )
                              P = ppl.tile([128, 512], BF16, tag="P",
                                           name=f"P{h}_{g}_{ktl}")
                              nc.scalar.activation(P, pss, AF.Exp, scale=0.125)
                              p_tiles[(h, g, ktl)] = P

                  def emit_attnv(w, g):
                      for h in range(w * 4, w * 4 + 4):
                          if g == 0:
                              psa[h] = psA.tile([65, 512], F32, tag="psA",
                                                name=f"psa{h}")
                          for ktl in range(4):
                              nc.tensor.matmul(
                                  psa[h],
                                  V_all[:, g, ktl, h, :],
                                  p_tiles.pop((h, g, ktl)),
                                  start=(g == 0 and ktl == 0),
                                  stop=(g == 3 and ktl == 3),
                              )
                          if g == 3:
                              nc.vector.tensor_copy(acc[:, h, :], psa.pop(h))

                  emit_scores(*steps[0])
                  for i in range(len(steps)):
                      if i + 1 < len(steps):
                          emit_scores(*steps[i + 1])
                      emit_attnv(*steps[i])

            # ---- Phase 4: softmax denominators + O proj + residual ----
            with (
                tc.tile_pool(name="attnp", bufs=1) as attnp,
                tc.tile_pool(name="dsm", bufs=2) as dsm,
                tc.tile_pool(name="xqp", bufs=1) as xqp,
                tc.tile_pool(name="dwo", bufs=8) as dwo,
                tc.tile_pool(name="dtmp", bufs=4) as dtmp,
                tc.tile_pool(name="psRB", bufs=2, space="PSUM") as psRB,
                tc.tile_pool(name="psO", bufs=4, space="PSUM") as psO,
            ):
                r16 = dsm.tile([16, 512], BF16)
                for h in range(H):
                    nc.sync.dma_start(out=r16[h:h + 1, :], in_=acc[64:65, h, :])
                rr = dsm.tile([16, 512], F32)
                nc.vector.reciprocal(rr, r16)
                rrb = dsm.tile([16, 512], BF16)
                nc.vector.tensor_copy(rrb, rr)
                # single-partition copy so each head's row is a base-0
                # moving operand for the rank-1 broadcast matmul
                rrow = dsm.tile([1, 16, 512], BF16)
                for h in range(H):
                    nc.sync.dma_start(out=rrow[0:1, h, :], in_=rrb[h:h + 1, :])

                attn128 = attnp.tile([128, 8, QT], BF16)
                for h in range(H):
                    rbps = psRB.tile([64, 512], F32, tag="rb")
                    nc.tensor.matmul(rbps, ones64, rrow[0:1, h, :],
                                     start=True, stop=True)
                    ko = (h % 2) * 64
                    nc.vector.tensor_mul(
                        attn128[ko:ko + 64, h // 2, :], acc[0:64, h, :], rbps
                    )

                xq_sb = xqp.tile([128, 4, D], F32)
                for st in range(4):
                    nc.sync.dma_start(
                        out=xq_sb[:, st, :],
                        in_=XQ[st * 128:(st + 1) * 128, :])
                for c in range(2):
                    po = [psO.tile([128, 512], F32, tag="psO",
                                   name=f"po{c}_{i}") for i in range(4)]
                    for j in range(8):
                        wot = dwo.tile([128, 512], BF16, tag="wo")
                        nc.sync.dma_start(
                            out=wot,
                            in_=WO[j * 128:(j + 1) * 128,
                                   c * 512:(c + 1) * 512],
                        )
                        for qt in range(4):
                            nc.tensor.matmul(
                                po[qt], attn128[:, j, qt * 128:(qt + 1) * 128],
                                wot, start=(j == 0), stop=(j == 7),
                            )
                    for qt in range(4):
                        t1 = dtmp.tile([128, 512], F32, tag="t1")
                        nc.vector.tensor_add(
                            t1, po[qt], bo_bc[:, c * 512:(c + 1) * 512]
                        )
                        nc.vector.tensor_add(
                            x2[:, qt, c * 512:(c + 1) * 512],
                            t1,
                            xq_sb[:, qt, c * 512:(c + 1) * 512],
                        )

            # ---- Phase 5: LN2 + MLP + residual ----
            with (
                tc.tile_pool(name="lnp2", bufs=3) as lnp2,
                tc.tile_pool(name="h2p", bufs=1) as h2p,
                tc.tile_pool(name="gp", bufs=1) as gp,
                tc.tile_pool(name="wfp", bufs=8) as wfp,
                tc.tile_pool(name="w2p", bufs=8) as w2p,
                tc.tile_pool(name="yp", bufs=2) as yp,
            ):
                h2T = h2p.tile([128, 8, QT], BF16)
                G = gp.tile([128, 32, QT], BF16)
                with (
                    tc.tile_pool(name="psT2", bufs=2, space="PSUM") as psT2,
                    tc.tile_pool(name="psF", bufs=4, space="PSUM") as psF,
                ):
                    # LN2 from SBUF-resident x2, transposed output
                    for st in range(4):
                        mr = _ln_stats(nc, lnp2, eps,
                                       x2[:, st, 0:512], x2[:, st, 512:1024])
                        h2 = lnp2.tile([128, D], BF16, tag="ln_h")
                        # h2 = x2*rstd + (-mu*rstd)
                        nc.vector.tensor_scalar(
                            h2, x2[:, st, :], mr[:, 1:2].bitcast(F32),
                            mr[:, 0:1].bitcast(F32), ALU.mult, ALU.add
                        )
                        for dt in range(8):
                            pst = psT2.tile([128, 128], BF16, tag="tp2")
                            nc.tensor.transpose(
                                pst, h2[:, dt * 128:(dt + 1) * 128], ident16
                            )
                            nc.vector.tensor_copy(
                                h2T[:, dt, st * 128:(st + 1) * 128], pst
                            )

                    # MLP1: gelu(h2 @ w1 + b1), transposed output [dff, q]
                    for ft in range(32):
                        w1c = wfp.tile([128, 8, 128], BF16, tag="w1")
                        nc.sync.dma_start(out=w1c, in_=W1R[ft])
                        psf = psF.tile([128, 512], F32, tag="psF")
                        for dt in range(8):
                            nc.tensor.matmul(
                                psf, w1c[:, dt, :], h2T[:, dt, :],
                                start=(dt == 0), stop=(dt == 7),
                            )
                        nc.scalar.activation(
                            G[:, ft, :], psf, AF.Gelu, bias=b1T[:, ft:ft + 1]
                        )

                # MLP2: y = G^T @ w2 + b2 + x2
                with tc.tile_pool(name="psY", bufs=4, space="PSUM") as psY:
                    for c in range(2):
                        py = [psY.tile([128, 512], F32, tag="psY",
                                       name=f"py{c}_{i}") for i in range(4)]
                        for ft in range(32):
                            w2t = w2p.tile([128, 512], BF16, tag="w2")
                            nc.sync.dma_start(
                                out=w2t,
                                in_=W2[ft * 128:(ft + 1) * 128,
                                       c * 512:(c + 1) * 512],
                            )
                            for qt in range(4):
                                nc.tensor.matmul(
                                    py[qt], G[:, ft, qt * 128:(qt + 1) * 128],
                                    w2t, start=(ft == 0), stop=(ft == 31),
                                )
                        for qt in range(4):
                            t1 = yp.tile([128, 512], F32, tag="yt1")
                            nc.vector.tensor_add(
                                t1, py[qt], b2_bc[:, c * 512:(c + 1) * 512]
                            )
                            yt = yp.tile([128, 512], F32, tag="yt2")
                            nc.vector.tensor_add(
                                yt, t1, x2[:, qt, c * 512:(c + 1) * 512]
                            )
                            nc.sync.dma_start(
                                out=Y[qt * 128:(qt + 1) * 128,
                                      c * 512:(c + 1) * 512],
                                in_=yt,
                            )

    nc.compile()
    return nc


_NC = None


def _get_nc():
    global _NC
    if _NC is None:
        _NC = _build()
    return _NC


def make_in_maps(inputs):
    f32 = lambda a: np.ascontiguousarray(np.asarray(a, dtype=np.float32))
    bf16 = lambda a: np.ascontiguousarray(
        np.asarray(a, dtype=np.float32).astype(ml_dtypes.bfloat16)
    )
    x = f32(inputs["x"])
    g1, b1l = f32(inputs["ln1_g"]), f32(inputs["ln1_b"])
    g2, b2l = f32(inputs["ln2_g"]), f32(inputs["ln2_b"])
    wq, wk, wv = f32(inputs["wq"]), f32(inputs["wk"]), f32(inputs["wv"])
    w1 = f32(inputs["w1"])

    # Fold LayerNorm affine params into the following projections (exact).
    wq_e = g1[:, None] * wq
    wk_e = g1[:, None] * wk
    wv_e = g1[:, None] * wv
    w1_e = g2[:, None] * w1

    # host pre-layouts: [out-block, partition, dt, n]
    def col_blocks(w, nblk, nsz):
        # w [1024, nblk*nsz] -> [nblk, 128, 8, nsz]
        return np.ascontiguousarray(
            w.reshape(8, 128, nblk, nsz).transpose(2, 1, 0, 3))

    common = {
        "wqr": bf16(col_blocks(wq_e, 8, 128)),
        "wkr": bf16(col_blocks(wk_e, 8, 128)),
        "wvr": bf16(col_blocks(wv_e, 2, 512)),
        "wo": bf16(inputs["wo"]),
        "w1r": bf16(col_blocks(w1_e, 32, 128)),
        "w2": bf16(inputs["w2"]),
        "bq": f32(inputs["bq"] + b1l @ wq),
        "bk": f32(inputs["bk"] + b1l @ wk),
        "bv": f32(inputs["bv"] + b1l @ wv),
        "bo": f32(inputs["bo"]),
        "b1": f32(inputs["b1"] + b2l @ w1),
        "b2": f32(inputs["b2"]),
    }
    in_maps = []
    for c in range(NCORES):
        b = c // 4
        qoff = (c % 4) * QT
        m = dict(common)
        m["xq"] = np.ascontiguousarray(x[b, qoff:qoff + QT])
        m["xqt"] = np.ascontiguousarray(x[b, qoff:qoff + QT].T)
        in_maps.append(m)
    return in_maps


def kernel(x, ln1_g, ln1_b, wq, bq, wk, bk, wv, bv, wo, bo, w1, b1, w2, b2,
           ln2_g, ln2_b):
    inputs = {
        "x": x, "ln1_g": ln1_g, "ln1_b": ln1_b,
        "wq": wq, "bq": bq, "wk": wk, "bk": bk, "wv": wv, "bv": bv,
        "wo": wo, "bo": bo, "w1": w1, "b1": b1, "w2": w2, "b2": b2,
        "ln2_g": ln2_g, "ln2_b": ln2_b,
    }
    in_maps = make_in_maps(inputs)
    nc = _get_nc()
    res = run_bass_kernel_spmd(nc, in_maps, core_ids=list(range(NCORES)))

    y = np.empty((B, S, D), dtype=np.float32)
    for c in range(NCORES):
        b = c // 4
        qoff = (c % 4) * QT
        y[b, qoff:qoff + QT] = res.results[c]["y"]
    return y
